# revision 5
# baseline (speedup 1.0000x reference)
"""Multi-head attention (B=4, S=2048, D=1024, H=16, causal, all-valid padding)
for 8 Trainium2 NeuronCores.

Sharding: hybrid data-parallel x tensor-parallel. Core c handles batch
b = c // 2 and head-group g = c % 2 (8 heads, 512 channels each). Each core
computes its head-group's Q/K/V projections, causal attention, and the
partial output projection through its slice of Wo. The host sums the two
head-group partials per batch (the row-parallel all-reduce) and stacks
batches.

On-chip layout (per core):
  - x fed pre-transposed (D, S) so D lands on partitions for the QKV matmuls.
  - Q^T, K^T kept as [128ch, S] tiles (two 64-ch heads stacked per pair) so
    scores are computed transposed: S^T[k,q] = K_tile @ Q^T, with the two
    heads of a pair row-packed into the PE array (dk=64 each).
  - P^T = exp(S^T/8) via one ACT instruction per k-tile (3D strided AP over
    both heads on diagonal tiles). The causal boundary is handled by one
    fused [128, 2, 128] bf16 multiply on the 128-wide diagonal strip only
    (the strip mask is the same lower-triangle for every diagonal tile).
  - ctx^T accumulates in PSUM via col-packed V-matmuls; softmax denominators
    accumulate pre-broadcast in a parallel bank via an all-ones stationary
    operand, so normalization is one reciprocal_approx_fast + one multiply.
  - Projection / output-projection matmuls are queued as small filler units
    (2 matmuls for QKV, 1 for Wo) and drip-fed between the attention matmuls
    of every k-tile so the PE array works through the exp (ACT) latency.
    QKV units are paced to finish inside their chunk (they gate the next
    chunk); Wo units are deferred as a reserve for the last, filler-poor
    chunk.
"""

import numpy as np
import ml_dtypes
from collections import deque

B, S, D, H = 4, 2048, 1024, 16
DK = D // H            # 64
CH = D // 2            # 512 local channels per core (8 heads)
NPAIR = 4              # pairs of heads per core (2 heads x 64ch = 128ch tile)
SCHUNK = 512           # s-chunk (q-chunk) width
KTILE = 128            # k-tile width
NDT = D // 128         # 8 d-tiles (contraction for projections)

_BF16 = ml_dtypes.bfloat16


def _build_nc(s_len):
    import concourse.bass as bass
    import concourse.mybir as mybir
    import concourse.tile as tile
    from concourse import bacc

    f32 = mybir.dt.float32
    bf16 = mybir.dt.bfloat16
    Exp = mybir.ActivationFunctionType.Exp

    nsc = s_len // SCHUNK          # s-chunks / q-chunks
    nkt_total = s_len // KTILE     # k-tiles

    nc = bacc.Bacc("TRN2", target_bir_lowering=False, debug=False)

    xq_d = nc.dram_tensor("xqT", [D, s_len], bf16, kind="ExternalInput")
    xk_d = nc.dram_tensor("xkT", [D, s_len], bf16, kind="ExternalInput")
    xv_d = nc.dram_tensor("xvT", [D, s_len], bf16, kind="ExternalInput")
    wq_d = nc.dram_tensor("wqT", [D, CH], bf16, kind="ExternalInput")
    wk_d = nc.dram_tensor("wkT", [D, CH], bf16, kind="ExternalInput")
    wv_d = nc.dram_tensor("wvT", [D, CH], bf16, kind="ExternalInput")
    wo_d = nc.dram_tensor("woT", [CH, D], bf16, kind="ExternalInput")
    mask_d = nc.dram_tensor("mask2", [128, 2, 128], bf16, kind="ExternalInput")
    y_d = nc.dram_tensor("y", [s_len, D], f32, kind="ExternalOutput")

    x_r = {
        "q": xq_d[:, :].rearrange("(d p) s -> p d s", p=128),
        "k": xk_d[:, :].rearrange("(d p) s -> p d s", p=128),
        "v": xv_d[:, :].rearrange("(d p) s -> p d s", p=128),
    }
    wq_r = wq_d[:, :].rearrange("(d p) c -> p d c", p=128)
    wk_r = wk_d[:, :].rearrange("(d p) c -> p d c", p=128)
    wv_r = wv_d[:, :].rearrange("(d p) c -> p d c", p=128)

    with tile.TileContext(nc) as tc:
        from contextlib import ExitStack

        with ExitStack() as ctx:
            const_pool = ctx.enter_context(tc.tile_pool(name="const", bufs=1))
            w_pool = ctx.enter_context(tc.tile_pool(name="weights", bufs=1))
            qt_pool = ctx.enter_context(tc.tile_pool(name="qt", bufs=NPAIR * nsc))
            kt_pool = ctx.enter_context(tc.tile_pool(name="kt", bufs=NPAIR * nsc))
            v_pool = ctx.enter_context(tc.tile_pool(name="v", bufs=nkt_total))
            ctx_pool = ctx.enter_context(tc.tile_pool(name="ctx", bufs=NPAIR * nsc))
            x_pool = ctx.enter_context(tc.tile_pool(name="x", bufs=6))
            pt_pool = ctx.enter_context(tc.tile_pool(name="pt", bufs=4))
            ev_pool = ctx.enter_context(tc.tile_pool(name="ev", bufs=4))
            y_pool = ctx.enter_context(tc.tile_pool(name="yout", bufs=3))
            qkv_ps = ctx.enter_context(
                tc.tile_pool(name="qkv_ps", bufs=2, space="PSUM"))
            st_ps = ctx.enter_context(
                tc.tile_pool(name="st_ps", bufs=2, space="PSUM"))
            ctx_ps_pool = ctx.enter_context(
                tc.tile_pool(name="ctx_ps", bufs=1, space="PSUM"))
            den_ps_pool = ctx.enter_context(
                tc.tile_pool(name="den_ps", bufs=1, space="PSUM"))

            # DMA order = first-use order: the very first projection group
            # only waits on one 256KB wq slice + half an xq chunk
            x_tiles = {}

            def one_x_dma(key, sc, split=False):
                t = x_pool.tile([128, NDT, SCHUNK], bf16, tag="x",
                                name=f"x{key}_{sc}")
                src = x_r[key][:, :, sc * SCHUNK:(sc + 1) * SCHUNK]
                if split:
                    h = NDT // 2
                    nc.sync.dma_start(t[:, :h, :], src[:, :h, :])
                    nc.sync.dma_start(t[:, h:, :], src[:, h:, :])
                else:
                    nc.sync.dma_start(t[:, :, :], src)
                x_tiles[(key, sc)] = t

            def issue_x_dma(sc):
                for key in ("q", "k", "v"):
                    one_x_dma(key, sc)

            wq_sb = []
            for m in range(NPAIR):
                t = w_pool.tile([128, NDT, 128], bf16, name=f"wq_{m}")
                wq_sb.append(t)
            nc.sync.dma_start(wq_sb[0][:, :, :], wq_r[:, :, 0:128])
            one_x_dma("q", 0, split=True)
            for m in range(1, NPAIR):
                nc.sync.dma_start(
                    wq_sb[m][:, :, :], wq_r[:, :, m * 128:(m + 1) * 128])
            wk_sb = w_pool.tile([128, NDT, CH], bf16)
            nc.sync.dma_start(wk_sb[:, :NDT // 2, :], wk_r[:, :NDT // 2, :])
            one_x_dma("k", 0, split=True)
            nc.sync.dma_start(wk_sb[:, NDT // 2:, :], wk_r[:, NDT // 2:, :])
            wv_sb = w_pool.tile([128, NDT, CH], bf16)
            nc.sync.dma_start(wv_sb[:, :NDT // 2, :], wv_r[:, :NDT // 2, :])
            one_x_dma("v", 0, split=True)
            nc.sync.dma_start(wv_sb[:, NDT // 2:, :], wv_r[:, NDT // 2:, :])
            mask_sb = const_pool.tile([128, 2, 128], bf16)
            nc.sync.dma_start(mask_sb[:, :, :], mask_d[:, :, :])
            ones_sb = const_pool.tile([128, 64], bf16)
            nc.vector.memset(ones_sb[:, :], 1.0)
            wo_sb = w_pool.tile([128, NPAIR, D], bf16)
            nc.sync.dma_start(
                wo_sb[:, :, :], wo_d[:, :].rearrange("(c p) o -> p c o", p=128))

            qt_tiles = {}
            kt_tiles = {}
            v_tiles = {}
            ctx_tiles = {}

            # two deferred-work queues of small PE filler units. qkv units
            # gate the next chunk (paced to finish in time); wo units have no
            # deadline and are held back for the filler-poor last chunk.
            fill_qkv = deque()
            fill_wo = deque()

            def pop_n(q, n):
                for _ in range(n):
                    if not q:
                        return
                    q.popleft()[1]()

            def flush_match(q, tag):
                rest = deque()
                while q:
                    item = q.popleft()
                    if item[0] == tag:
                        item[1]()
                    else:
                        rest.append(item)
                q.extend(rest)

            def qkv_units(kind, m, sc):
                """One projection group (8 accumulating matmuls + PSUM->SBUF
                copy) split into 4 filler units of 2 matmuls each."""
                state = {}

                def mk(d0, d1):
                    def emit():
                        if "ps" not in state:
                            w = SCHUNK if kind != "v" else CH
                            state["ps"] = qkv_ps.tile(
                                [128, w], f32, tag="qkv",
                                name=f"qkvps_{kind}_{m}_{sc}")
                        ps = state["ps"]
                        xt = x_tiles[(kind, sc)]
                        for d in range(d0, d1):
                            if kind == "q":
                                nc.tensor.matmul(
                                    ps[:, :], lhsT=wq_sb[m][:, d, :],
                                    rhs=xt[:, d, :],
                                    start=(d == 0), stop=(d == NDT - 1))
                            elif kind == "k":
                                nc.tensor.matmul(
                                    ps[:, :],
                                    lhsT=wk_sb[:, d, m * 128:(m + 1) * 128],
                                    rhs=xt[:, d, :],
                                    start=(d == 0), stop=(d == NDT - 1))
                            else:
                                nc.tensor.matmul(
                                    ps[:, :],
                                    lhsT=xt[:, d, m * 128:(m + 1) * 128],
                                    rhs=wv_sb[:, d, :],
                                    start=(d == 0), stop=(d == NDT - 1))
                        if d1 == NDT:
                            if kind == "q":
                                t = qt_pool.tile([128, SCHUNK], bf16, tag="qt",
                                                 name=f"qt_{m}_{sc}")
                                nc.vector.tensor_copy(t[:, :], ps[:, :])
                                qt_tiles[(m, sc)] = t
                            elif kind == "k":
                                t = kt_pool.tile([128, SCHUNK], bf16, tag="kt",
                                                 name=f"kt_{m}_{sc}")
                                nc.vector.tensor_copy(t[:, :], ps[:, :])
                                kt_tiles[(m, sc)] = t
                            else:
                                kt_idx = sc * (SCHUNK // 128) + m
                                t = v_pool.tile([128, CH], bf16, tag="v",
                                                name=f"v_{kt_idx}")
                                nc.vector.tensor_copy(t[:, :], ps[:, :])
                                v_tiles[kt_idx] = t
                    return emit

                return [mk(d, d + 2) for d in range(0, NDT, 2)]

            def push_qkv(sc):
                # q/k of pair 0 and all v first: they unblock the first pair
                # of the next chunk's attention
                order = [("q", 0), ("k", 0), ("v", 0), ("v", 1), ("v", 2),
                         ("v", 3)]
                order += [(k, m) for m in range(1, NPAIR)
                          for k in ("q", "k")]
                for kind, m in order:
                    tag = ("qkv", sc, "v" if kind == "v" else (kind, m))
                    for u in qkv_units(kind, m, sc):
                        fill_qkv.append((tag, u))

            def wo_units(qt, oc, qc):
                """One output-projection group (4 matmuls + copy + DMA) as
                4 single-matmul filler units."""
                state = {}

                def mk(cj):
                    def emit():
                        if "ps" not in state:
                            state["ps"] = qkv_ps.tile(
                                [128, 512], f32, tag="qkv",
                                name=f"wops_{qt}_{oc}")
                        ps = state["ps"]
                        nc.tensor.matmul(
                            ps[:, :],
                            lhsT=ctx_tiles[(cj, qc)][:, (qt % 4) * 128:
                                                     (qt % 4 + 1) * 128],
                            rhs=wo_sb[:, cj, oc * 512:(oc + 1) * 512],
                            start=(cj == 0), stop=(cj == NPAIR - 1))
                        if cj == NPAIR - 1:
                            yt = y_pool.tile([128, 512], f32, tag="yout")
                            nc.vector.tensor_copy(yt[:, :], ps[:, :])
                            nc.sync.dma_start(
                                y_d[qt * 128:(qt + 1) * 128,
                                    oc * 512:(oc + 1) * 512],
                                yt[:, :])
                    return emit

                return [mk(cj) for cj in range(NPAIR)]

            def push_wo(qc):
                for qt in range(qc * 4, (qc + 1) * 4):
                    for oc in range(D // 512):
                        for u in wo_units(qt, oc, qc):
                            fill_wo.append((("wo", qc), u))

            push_qkv(0)
            for sc in range(nsc):
                if sc + 1 < nsc:
                    issue_x_dma(sc + 1)
                    push_qkv(sc + 1)

                # ---- attention for q-chunk qc = sc ----
                qc = sc
                nkt = (qc + 1) * (SCHUNK // KTILE)  # causal: k-tiles 0..nkt-1
                tiles_left = NPAIR * nkt
                for pair in range(NPAIR):
                    # correctness deadline: this pair's q/k (and v for the
                    # whole chunk) must exist before its attention
                    flush_match(fill_qkv, ("qkv", qc, ("q", pair)))
                    flush_match(fill_qkv, ("qkv", qc, ("k", pair)))
                    if pair == 0:
                        flush_match(fill_qkv, ("qkv", qc, "v"))

                    ctx_p = ctx_ps_pool.tile([128, SCHUNK], f32, tag="ctxps")
                    den_p = den_ps_pool.tile([128, SCHUNK], f32, tag="denps")

                    def tile_off(kt):
                        # diagonal tile r: columns [0, 128r) are fully masked
                        r = kt - qc * (SCHUNK // KTILE)
                        return 128 * r if r > 0 else 0

                    def emit_scores(kt):
                        off = tile_off(kt)
                        st = st_ps.tile([128, 2, SCHUNK], f32, tag="st")
                        ktile = kt_tiles[(pair, kt // 4)]
                        qtile = qt_tiles[(pair, qc)]
                        for h in range(2):
                            nc.tensor.matmul(
                                st[:, h, off:],
                                lhsT=ktile[h * 64:(h + 1) * 64,
                                           (kt % 4) * KTILE:(kt % 4 + 1) * KTILE],
                                rhs=qtile[h * 64:(h + 1) * 64, off:],
                                start=True, stop=True)
                        pt = pt_pool.tile([128, 2, SCHUNK], bf16, tag="pt")
                        # one exp per k-tile: 3D strided AP covers both heads
                        nc.scalar.activation(pt[:, :, off:], st[:, :, off:],
                                             Exp, scale=0.125)
                        r = kt - qc * (SCHUNK // KTILE)
                        if r >= 0:
                            # causal boundary: the 128-wide diagonal strip is
                            # the same lower-triangle mask for every r
                            nc.vector.tensor_mul(
                                pt[:, :, off:off + 128],
                                pt[:, :, off:off + 128],
                                mask_sb[:, :, :])
                        return pt

                    pt_cur = emit_scores(0)
                    for kt in range(nkt):
                        pt_next = emit_scores(kt + 1) if kt + 1 < nkt else None
                        # paced PE filler: cover the exp (ACT) wait before the
                        # ctx/den matmuls become runnable. qkv is deadline-
                        # driven; wo is the reserve when qkv runs dry.
                        nq = -(-len(fill_qkv) // max(tiles_left, 1))
                        if nq:
                            pop_n(fill_qkv, min(nq, 8))
                        elif fill_wo:
                            pop_n(fill_wo, 2)
                        tiles_left -= 1
                        off = tile_off(kt)
                        vt = v_tiles[kt]
                        for h in range(2):
                            hl = pair * 2 + h
                            nc.tensor.matmul(
                                ctx_p[h * 64:(h + 1) * 64, off:],
                                lhsT=vt[:, hl * 64:(hl + 1) * 64],
                                rhs=pt_cur[:, h, off:],
                                start=(kt == 0), stop=(kt == nkt - 1),
                                tile_position=(0, h * 64),
                                skip_group_check=True)
                            nc.tensor.matmul(
                                den_p[h * 64:(h + 1) * 64, off:],
                                lhsT=ones_sb[:, :],
                                rhs=pt_cur[:, h, off:],
                                start=(kt == 0), stop=(kt == nkt - 1),
                                tile_position=(0, h * 64),
                                skip_group_check=True)
                        pt_cur = pt_next

                    # normalization: denominators arrive pre-broadcast across
                    # each head's 64 partitions; one recip + one multiply
                    rec = ev_pool.tile([128, SCHUNK], f32, tag="rec")
                    nc.vector.reciprocal_approx_fast(rec[:, :], den_p[:, :])
                    t = ctx_pool.tile([128, SCHUNK], bf16, tag="ctx",
                                      name=f"ctx_{pair}_{qc}")
                    nc.vector.tensor_mul(t[:, :], ctx_p[:, :], rec[:, :])
                    ctx_tiles[(pair, qc)] = t

                push_wo(qc)
            while fill_qkv:
                fill_qkv.popleft()[1]()
            while fill_wo:
                fill_wo.popleft()[1]()

    nc.finalize()
    return nc


def _make_mask2():
    ki = np.arange(128)[:, None]
    ji = np.arange(128)[None, :]
    strip = (ji >= ki)
    return np.stack([strip, strip], axis=1).astype(_BF16)  # [128, 2, 128]


def _host_shards(x_query, x_key, x_value, Wq, Wk, Wv, Wo, s_len):
    """Per-core input dicts. Core c: batch c//2, head-group c%2."""
    mask2 = _make_mask2()
    in_maps = []
    for c in range(8):
        b, g = c // 2, c % 2
        lo, hi = g * CH, (g + 1) * CH
        in_maps.append({
            "xqT": np.ascontiguousarray(x_query[b, :s_len].T).astype(_BF16),
            "xkT": np.ascontiguousarray(x_key[b, :s_len].T).astype(_BF16),
            "xvT": np.ascontiguousarray(x_value[b, :s_len].T).astype(_BF16),
            "wqT": np.ascontiguousarray(Wq[lo:hi, :].T).astype(_BF16),
            "wkT": np.ascontiguousarray(Wk[lo:hi, :].T).astype(_BF16),
            "wvT": np.ascontiguousarray(Wv[lo:hi, :].T).astype(_BF16),
            "woT": np.ascontiguousarray(Wo[:, lo:hi].T).astype(_BF16),
            "mask2": mask2,
        })
    return in_maps


_NC_CACHE = {}


def _get_nc(s_len):
    if s_len not in _NC_CACHE:
        _NC_CACHE[s_len] = _build_nc(s_len)
    return _NC_CACHE[s_len]


def kernel(x_query, x_key, x_value, attention_mask, Wq, Wk, Wv, Wo,
           _trace=False):
    from concourse.bass_utils import run_bass_kernel_spmd

    nc = _get_nc(S)
    in_maps = _host_shards(x_query, x_key, x_value, Wq, Wk, Wv, Wo, S)
    res = run_bass_kernel_spmd(nc, in_maps, core_ids=list(range(8)),
                               trace=_trace)
    y = np.empty((B, S, D), dtype=np.float32)
    for b in range(B):
        y[b] = res.results[2 * b]["y"].astype(np.float32) + \
            res.results[2 * b + 1]["y"].astype(np.float32)
    if _trace:
        return y, res
    return y
